# revision 9
# baseline (speedup 1.0000x reference)
"""MHA kernel for trn2, 8 NeuronCores, head-sharded (2 heads/core).

Per core c (heads 2c, 2c+1):
  qT/kT = (w_{q,k} shard).T @ x.T  -> [128, T] bf16 (rows 0:64 head a, 64:128 head b)
  vT    = (w_v shard).T @ x.T      -> [128, T] bf16, then DMA-xbar-transposed
          into Vp [128 keys, KB, 2 heads, 65] (col 64 = ones row for Z)
  per q-tile (512) x key-block (128):
     S^T = kT_blk.T @ qT   (row-tiled pair, K=64 per head, one [128,1024] psum)
     eS  = exp(S^T)        75% of blocks on ACT (exact exp, q pre-scaled by
           1/8 on host); 25% on DVE via a bf16 Schraudolph bit-trick
           (int16(S*128*log2e + (127-c)*128) reinterpreted as bf16) --
           the multiplicative bias cancels in softmax normalization.
     y'[65,512] += Vp[kb,h].T @ eS   (row 64 = Z = sum exp)
  y0s/y1s = bf16 casts of y'[0:64]; Z rows -> DRAM
  out-proj per head (K=64): outT{0,1}[:, qtile] = wo[head rows].T @ y{0,1}s
  (out-proj of q-tile i is emitted inside q-tile i+1's loop to hide latency)

Host: normalizes per-head partials by Z and sums over cores/heads.
"""

import numpy as np
import ml_dtypes

import concourse.bacc as bacc
import concourse.mybir as mybir
from concourse.tile import TileContext
from concourse.bass_utils import run_bass_kernel_spmd

BF16 = ml_dtypes.bfloat16
F32 = mybir.dt.float32
BF = mybir.dt.bfloat16
I16 = mybir.dt.int16
EXP = mybir.ActivationFunctionType.Exp
MULT = mybir.AluOpType.mult
ADD = mybir.AluOpType.add

B, T, C = 1, 4096, 1024
H, D = 16, 64
NCORES = 8
P = 128
CB = C // P          # 8 contraction blocks
KB = T // P          # 32 key blocks
QTS = T // 512       # 8 q tiles

# Schraudolph bf16 exp trick: bits = round(S * 128*log2(e) + (127 - c)*128)
LOG2E = 1.4426950408889634
TRICK_SCALE = 128.0 * LOG2E
TRICK_BIAS = (127.0 - 0.0579) * 128.0
TRICK_EVERY = 4      # every 4th (qt,kb) block's exp runs on DVE instead of ACT
TRICK_PHASE = 1

_cached = None


def build_bass():
    global _cached
    if _cached is not None:
        return _cached

    nc = bacc.Bacc("TRN2", target_bir_lowering=False, name="mha_head_sharded")

    xT = nc.dram_tensor("xT", (C, T), BF, kind="ExternalInput")
    wq = nc.dram_tensor("wq", (C, P), BF, kind="ExternalInput")
    wk = nc.dram_tensor("wk", (C, P), BF, kind="ExternalInput")
    wv = nc.dram_tensor("wv", (C, P), BF, kind="ExternalInput")
    wo = nc.dram_tensor("wo", (P, C), BF, kind="ExternalInput")
    outT0 = nc.dram_tensor("outT0", (C, T), BF, kind="ExternalOutput")
    outT1 = nc.dram_tensor("outT1", (C, T), BF, kind="ExternalOutput")
    Zt = nc.dram_tensor("Zt", (2, T), F32, kind="ExternalOutput")

    with TileContext(nc) as tc:
        with (
            tc.tile_pool(name="const", bufs=1) as const,
            tc.tile_pool(name="work", bufs=3) as work,
            tc.tile_pool(name="psS", bufs=2, space="PSUM") as psS,
            tc.tile_pool(name="psY", bufs=1, space="PSUM") as psY,
            tc.tile_pool(name="psO", bufs=2, space="PSUM") as psO,
        ):
            # ---- load inputs (weights on gpsimd queue, x split across queues) ----
            wqs = const.tile([P, CB, P], BF)
            nc.gpsimd.dma_start(wqs[:], wq[:, :].rearrange("(cb p) f -> p cb f", p=P))
            wks = const.tile([P, CB, P], BF)
            nc.gpsimd.dma_start(wks[:], wk[:, :].rearrange("(cb p) f -> p cb f", p=P))
            wvs = const.tile([P, CB, P], BF)
            nc.gpsimd.dma_start(wvs[:], wv[:, :].rearrange("(cb p) f -> p cb f", p=P))
            wos = const.tile([P, C], BF)
            nc.gpsimd.dma_start(wos[:], wo[:, :])
            xTs = const.tile([P, CB, T], BF)
            xT_r = xT[:, :].rearrange("(cb p) t -> p cb t", p=P)
            for tt in range(QTS):
                q = nc.sync if tt % 2 == 0 else nc.gpsimd
                q.dma_start(xTs[:, :, tt * 512:(tt + 1) * 512],
                            xT_r[:, :, tt * 512:(tt + 1) * 512])

            # ---- warm up the PE clock (HAM) during the input DMA wait; also
            # preload the exp table set on ACT so the first real exp is fast ----
            warm = const.tile([P, 512], BF)
            nc.vector.memset(warm[:], 0.0)
            warm_e = work.tile([P, 16], BF, tag="we")
            nc.scalar.activation(warm_e[:], warm[:, 0:16], EXP)
            for _ in range(30):
                pw = psO.tile([P, 512], F32, tag="po", name="pw")
                nc.tensor.matmul(pw[:, :], warm[:, 0:P], warm[:, :],
                                 start=True, stop=True)

            qTs = const.tile([P, T], BF)
            kTs = const.tile([P, T], BF)
            vTs = const.tile([P, T], BF)
            # Vp{A,B}[key, kb, 0:64] = V head {a,b}, col 64 = ones (Z row)
            VpA = const.tile([P, KB, 80], BF)
            VpB = const.tile([P, KB, 80], BF)
            nc.vector.memset(VpA[:, :, 64:65], 1.0)
            nc.vector.memset(VpB[:, :, 64:65], 1.0)

            # ---- S^T + exp emission helper (used by phase pre-lag + main loop) ----
            eS_store = {}

            def emit_s_exp(qt, kb):
                q0 = qt * 512
                k0 = kb * P
                s = psS.tile([P, 1024], F32, tag="s", name="s")
                nc.tensor.matmul(s[:, 0:512], kTs[0:64, k0:k0 + P],
                                 qTs[0:64, q0:q0 + 512], start=True, stop=True)
                nc.tensor.matmul(s[:, 512:1024], kTs[64:128, k0:k0 + P],
                                 qTs[64:128, q0:q0 + 512], start=True, stop=True)
                eS = work.tile([P, 1024], BF, tag="es", bufs=17)
                if (qt * KB + kb) % TRICK_EVERY == TRICK_PHASE:
                    nc.vector.tensor_scalar(
                        eS[:].bitcast(I16), s[:], TRICK_SCALE, TRICK_BIAS,
                        MULT, ADD)
                else:
                    nc.scalar.activation(eS[:], s[:], EXP)
                eS_store[(qt, kb)] = eS

            LAG = 14

            # ---- kT + qT(first tile) + vT, interleaved per tok tile so the
            # PE chases the arriving xT DMA chunks ----
            for tt in range(QTS):
                t0 = tt * 512
                pq = psS.tile([P, 1024], F32, tag="s")
                for cb in range(CB):
                    nc.tensor.matmul(
                        pq[:, 0:512], wks[:, cb, :], xTs[:, cb, t0:t0 + 512],
                        start=(cb == 0), stop=(cb == CB - 1))
                if tt == 0:
                    for cb in range(CB):
                        nc.tensor.matmul(
                            pq[:, 512:1024], wqs[:, cb, :], xTs[:, cb, 0:512],
                            start=(cb == 0), stop=(cb == CB - 1))
                nc.vector.tensor_copy(kTs[:, t0:t0 + 512], pq[:, 0:512])
                if tt == 0:
                    nc.vector.tensor_copy(qTs[:, 0:512], pq[:, 512:1024])
                # vT chunk: wv-stationary, accumulate over cb in one psum tile
                pv = psO.tile([P, 512], F32, tag="po", name="pv")
                for cb in range(CB):
                    nc.tensor.matmul(
                        pv[:, :], wvs[:, cb, :], xTs[:, cb, t0:t0 + 512],
                        start=(cb == 0), stop=(cb == CB - 1))
                nc.vector.tensor_copy(vTs[:, t0:t0 + 512], pv[:])
                # xbar-transpose each 128-key block into VpA/VpB (per head)
                for kb in range(tt * 4, tt * 4 + 4):
                    nc.sync.dma_start_transpose(
                        VpA[:, kb, 0:64], vTs[0:64, kb * P:(kb + 1) * P])
                    nc.sync.dma_start_transpose(
                        VpB[:, kb, 0:64], vTs[64:128, kb * P:(kb + 1) * P])
                # pre-emit the first S^T+exp pairs of q-tile 0 so ACT starts early
                if tt >= 1:
                    for kb_pre in range((tt - 1) * 2, (tt - 1) * 2 + 2):
                        emit_s_exp(0, kb_pre)

            def emit_qt_proj(tokt):
                # compute qT for tok tile `tokt` using a shared psum slot
                pqd = psO.tile([P, 512], F32, tag="po", name="pqd")
                for cb in range(CB):
                    nc.tensor.matmul(
                        pqd[:, :], wqs[:, cb, :], xTs[:, cb, tokt * 512:(tokt + 1) * 512],
                        start=(cb == 0), stop=(cb == CB - 1))
                nc.vector.tensor_copy(qTs[:, tokt * 512:(tokt + 1) * 512], pqd[:])

            # ---- main loop: attention with deferred out-projection ----
            def emit_outproj(dep, fb, flush=False):
                # row-tiled concurrent pair: head a on array rows 0:64, head b on 64:128
                yns, q0 = dep
                if flush and fb % 2 == 1:
                    st = psS.tile([P, 1024], F32, tag="s", name="st")
                    poA, poB = st[:, 0:512], st[:, 512:1024]
                else:
                    poA = psO.tile([P, 512], F32, tag="po", name="poA")
                    poB = psO.tile([P, 512], F32, tag="po", name="poB")
                nc.tensor.matmul(poA[:, :], wos[0:64, fb * P:(fb + 1) * P],
                                 yns[0:64, :], start=True, stop=True)
                nc.tensor.matmul(poB[:, :], wos[64:128, fb * P:(fb + 1) * P],
                                 yns[64:128, :], start=True, stop=True)
                ocA = work.tile([P, 512], BF, tag="oc")
                nc.vector.tensor_copy(ocA[:], poA[:])
                nc.gpsimd.dma_start(outT0[fb * P:(fb + 1) * P, q0:q0 + 512], ocA[:])
                ocB = work.tile([P, 512], BF, tag="oc")
                nc.vector.tensor_copy(ocB[:], poB[:])
                nc.sync.dma_start(outT1[fb * P:(fb + 1) * P, q0:q0 + 512], ocB[:])

            pending = None
            for qt in range(QTS):
                q0 = qt * 512
                y0 = psY.tile([65, 512], F32, tag="y0")
                y1 = psY.tile([65, 512], F32, tag="y1")
                for kb in range(KB):
                    tgt = qt * KB + kb + LAG
                    if tgt < QTS * KB:
                        emit_s_exp(tgt // KB, tgt % KB)
                    eS = eS_store.pop((qt, kb))
                    nc.tensor.matmul(y0[:, :], VpA[:, kb, 0:65], eS[:, 0:512],
                                     start=(kb == 0), stop=(kb == KB - 1))
                    nc.tensor.matmul(y1[:, :], VpB[:, kb, 0:65], eS[:, 512:1024],
                                     start=(kb == 0), stop=(kb == KB - 1))
                    if pending is not None and kb % 4 == 1:
                        emit_outproj(pending, kb // 4)
                    if kb == 8 and qt + 1 < QTS:
                        emit_qt_proj(qt + 1)

                # casts release Y psum; Z rows go straight to DRAM.
                # yns rows 64:128 (head b) arrive via SBUF->SBUF DMA partition move.
                yns = work.tile([P, 512], BF, tag="yns", bufs=2)
                y1t = work.tile([64, 512], BF, tag="y1t")
                nc.vector.tensor_copy(yns[0:64, :], y0[0:64, :])
                nc.vector.tensor_copy(y1t[:], y1[0:64, :])
                nc.gpsimd.dma_start(yns[64:128, :], y1t[:])
                ztile = work.tile([65, 1024], F32, tag="zt")
                nc.vector.tensor_copy(ztile[64:65, 0:512], y0[64:65, :])
                nc.vector.tensor_copy(ztile[64:65, 512:1024], y1[64:65, :])
                nc.gpsimd.dma_start(Zt[0:1, q0:q0 + 512], ztile[64:65, 0:512])
                nc.gpsimd.dma_start(Zt[1:2, q0:q0 + 512], ztile[64:65, 512:1024])
                pending = (yns, q0)

            for fb in range(CB):
                emit_outproj(pending, fb, flush=True)

    nc.compile()
    _cached = nc
    return nc


def make_in_maps(x, w_qkv, w_out):
    """x [1,T,C] f32, w_qkv [C, 3C] f32, w_out [C, C] f32 -> per-core input dicts."""
    x = np.asarray(x, dtype=np.float32)
    w_qkv = np.asarray(w_qkv, dtype=np.float32)
    w_out = np.asarray(w_out, dtype=np.float32)
    scale = 1.0 / np.sqrt(np.float32(D))
    xT = np.ascontiguousarray(x.reshape(T, C).T).astype(BF16)  # [C, T]
    in_maps = []
    for c in range(NCORES):
        cols = slice(P * c, P * (c + 1))
        wq = np.ascontiguousarray(w_qkv[:, 0:C][:, cols] * scale).astype(BF16)
        wk = np.ascontiguousarray(w_qkv[:, C:2 * C][:, cols]).astype(BF16)
        wv = np.ascontiguousarray(w_qkv[:, 2 * C:3 * C][:, cols]).astype(BF16)
        wo = np.ascontiguousarray(w_out[P * c:P * (c + 1), :]).astype(BF16)
        in_maps.append({"xT": xT, "wq": wq, "wk": wk, "wv": wv, "wo": wo})
    return in_maps


def run(x, w_qkv, w_out, trace=False):
    nc = build_bass()
    in_maps = make_in_maps(x, w_qkv, w_out)
    res = run_bass_kernel_spmd(nc, in_maps, core_ids=list(range(NCORES)), trace=trace)
    acc = np.zeros((C, T), dtype=np.float32)
    for r in res.results:
        z = r["Zt"]  # [2, T]
        acc += r["outT0"].astype(np.float32) / z[0][None, :]
        acc += r["outT1"].astype(np.float32) / z[1][None, :]
    out = np.ascontiguousarray(acc.T).reshape(B, T, C)
    return out, res


def kernel(x, w_qkv, w_out):
    out, _ = run(x, w_qkv, w_out, trace=False)
    return out


# revision 16
# speedup vs baseline: 1.1014x; 1.1014x over previous
"""MHA kernel for trn2, 8 NeuronCores, head-sharded (2 heads/core).

Per core c (heads 2c, 2c+1):
  qT/kT = (w_{q,k} shard).T @ x.T  -> [128, T] bf16 (rows 0:64 head a, 64:128 head b)
  vT    = (w_v shard).T @ x.T      -> [128, T] bf16, then DMA-xbar-transposed
          into Vp [128 keys, KB, 2 heads, 65] (col 64 = ones row for Z)
  per q-tile (512) x key-block (128):
     S^T = kT_blk.T @ qT   (row-tiled pair, K=64 per head, one [128,1024] psum)
     eS  = exp(S^T)        75% of blocks on ACT (exact exp, q pre-scaled by
           1/8 on host); 25% on DVE via a bf16 Schraudolph bit-trick
           (int16(S*128*log2e + (127-c)*128) reinterpreted as bf16) --
           the multiplicative bias cancels in softmax normalization.
     y'[65,512] += Vp[kb,h].T @ eS   (row 64 = Z = sum exp)
  y0s/y1s = bf16 casts of y'[0:64]; Z rows -> DRAM
  out-proj per head (K=64): outT{0,1}[:, qtile] = wo[head rows].T @ y{0,1}s
  (out-proj of q-tile i is emitted inside q-tile i+1's loop to hide latency)

Host: normalizes per-head partials by Z and sums over cores/heads.
"""

import numpy as np
import ml_dtypes

import concourse.bacc as bacc
import concourse.mybir as mybir
from concourse.tile import TileContext
from concourse.bass_utils import run_bass_kernel_spmd

BF16 = ml_dtypes.bfloat16
F32 = mybir.dt.float32
BF = mybir.dt.bfloat16
I16 = mybir.dt.int16
EXP = mybir.ActivationFunctionType.Exp
MULT = mybir.AluOpType.mult
ADD = mybir.AluOpType.add

B, T, C = 1, 4096, 1024
H, D = 16, 64
NCORES = 8
P = 128
CB = C // P          # 8 contraction blocks
KB = T // P          # 32 key blocks
QTS = T // 512       # 8 q tiles

# Schraudolph bf16 exp trick: bits = round(S * 128*log2(e) + (127 - c)*128)
LOG2E = 1.4426950408889634
TRICK_SCALE = 128.0 * LOG2E
TRICK_BIAS = (127.0 - 0.0579) * 128.0
TRICK_EVERY = 4      # every 4th (qt,kb) block's exp runs on DVE instead of ACT
TRICK_PHASE = 1

_cached = None


def build_bass():
    global _cached
    if _cached is not None:
        return _cached

    nc = bacc.Bacc("TRN2", target_bir_lowering=False, name="mha_head_sharded")

    xT = nc.dram_tensor("xT", (C, T), BF, kind="ExternalInput")
    wq = nc.dram_tensor("wq", (C, P), BF, kind="ExternalInput")
    wk = nc.dram_tensor("wk", (C, P), BF, kind="ExternalInput")
    wv = nc.dram_tensor("wv", (C, P), BF, kind="ExternalInput")
    wo = nc.dram_tensor("wo", (P, C), BF, kind="ExternalInput")
    ident = nc.dram_tensor("ident", (P, P), BF, kind="ExternalInput")
    outT0 = nc.dram_tensor("outT0", (C, T), BF, kind="ExternalOutput")
    outT1 = nc.dram_tensor("outT1", (C, T), BF, kind="ExternalOutput")
    Zt = nc.dram_tensor("Zt", (2, T), F32, kind="ExternalOutput")

    with TileContext(nc) as tc:
        with (
            tc.tile_pool(name="const", bufs=1) as const,
            tc.tile_pool(name="work", bufs=3) as work,
            tc.tile_pool(name="psS", bufs=2, space="PSUM") as psS,
            tc.tile_pool(name="psY", bufs=1, space="PSUM") as psY,
            tc.tile_pool(name="psO", bufs=2, space="PSUM") as psO,
        ):
            # ---- load inputs (weights on gpsimd queue, x split across queues) ----
            wqs = const.tile([P, CB, P], BF)
            nc.gpsimd.dma_start(wqs[:], wq[:, :].rearrange("(cb p) f -> p cb f", p=P))
            wks = const.tile([P, CB, P], BF)
            nc.gpsimd.dma_start(wks[:], wk[:, :].rearrange("(cb p) f -> p cb f", p=P))
            wvs = const.tile([P, CB, P], BF)
            nc.gpsimd.dma_start(wvs[:], wv[:, :].rearrange("(cb p) f -> p cb f", p=P))
            wos = const.tile([P, C], BF)
            nc.gpsimd.dma_start(wos[:], wo[:, :])
            idn = const.tile([P, P], BF)
            nc.gpsimd.dma_start(idn[:], ident[:, :])
            xTs = const.tile([P, CB, T], BF)
            xT_r = xT[:, :].rearrange("(cb p) t -> p cb t", p=P)
            for tt in range(QTS):
                q = nc.sync if tt % 2 == 0 else nc.gpsimd
                q.dma_start(xTs[:, :, tt * 512:(tt + 1) * 512],
                            xT_r[:, :, tt * 512:(tt + 1) * 512])

            # ---- warm up the PE clock (HAM) during the input DMA wait; also
            # preload the exp table set on ACT so the first real exp is fast ----
            warm = const.tile([P, 512], BF)
            nc.vector.memset(warm[:], 0.0)
            warm_e = work.tile([P, 16], BF, tag="we")
            nc.scalar.activation(warm_e[:], warm[:, 0:16], EXP)
            for _ in range(30):
                pw = psO.tile([P, 512], F32, tag="po", name="pw")
                nc.tensor.matmul(pw[:, :], warm[:, 0:P], warm[:, :],
                                 start=True, stop=True)

            qTs = const.tile([P, T], BF)
            kTs = const.tile([P, T], BF)
            vTs = const.tile([P, T], BF)
            # Vp[key, kb, head, 0:64] = V, col 64 = ones (Z row); pad to 80
            Vp = const.tile([P, KB, 2, 80], BF)
            nc.vector.memset(Vp[:, :, :, 64:65], 1.0)

            # ---- S^T + exp emission helper (used by phase pre-lag + main loop) ----
            eS_store = {}

            def emit_s_exp(qt, kb):
                q0 = qt * 512
                k0 = kb * P
                s = psS.tile([P, 1024], F32, tag="s", name="s")
                nc.tensor.matmul(s[:, 0:512], kTs[0:64, k0:k0 + P],
                                 qTs[0:64, q0:q0 + 512], start=True, stop=True)
                nc.tensor.matmul(s[:, 512:1024], kTs[64:128, k0:k0 + P],
                                 qTs[64:128, q0:q0 + 512], start=True, stop=True)
                eS = work.tile([P, 1024], BF, tag="es", bufs=17)
                if (qt * KB + kb) % TRICK_EVERY == TRICK_PHASE:
                    nc.vector.tensor_scalar(
                        eS[:].bitcast(I16), s[:], TRICK_SCALE, TRICK_BIAS,
                        MULT, ADD)
                else:
                    nc.scalar.activation(eS[:], s[:], EXP)
                eS_store[(qt, kb)] = eS

            LAG = 14

            # ---- kT + qT(first tile) + vT, interleaved per tok tile so the
            # PE chases the arriving xT DMA chunks ----
            for tt in range(QTS):
                t0 = tt * 512
                pq = psS.tile([P, 1024], F32, tag="s")
                for cb in range(CB):
                    nc.tensor.matmul(
                        pq[:, 0:512], wks[:, cb, :], xTs[:, cb, t0:t0 + 512],
                        start=(cb == 0), stop=(cb == CB - 1))
                if tt == 0:
                    for cb in range(CB):
                        nc.tensor.matmul(
                            pq[:, 512:1024], wqs[:, cb, :], xTs[:, cb, 0:512],
                            start=(cb == 0), stop=(cb == CB - 1))
                nc.vector.tensor_copy(kTs[:, t0:t0 + 512], pq[:, 0:512])
                if tt == 0:
                    nc.vector.tensor_copy(qTs[:, 0:512], pq[:, 512:1024])
                # vT chunk: wv-stationary, accumulate over cb in one psum tile
                pv = psO.tile([P, 512], F32, tag="po", name="pv")
                for cb in range(CB):
                    nc.tensor.matmul(
                        pv[:, :], wvs[:, cb, :], xTs[:, cb, t0:t0 + 512],
                        start=(cb == 0), stop=(cb == CB - 1))
                nc.vector.tensor_copy(vTs[:, t0:t0 + 512], pv[:])
                # PE-transpose each 128-key block into Vp (both heads at once)
                for kb in range(tt * 4, tt * 4 + 4):
                    pt = psO.tile([P, P], BF, tag="po", name="pt")
                    nc.tensor.transpose(pt[:], vTs[:, kb * P:(kb + 1) * P], idn[:])
                    nc.vector.tensor_copy(Vp[:, kb, :, 0:64], pt[:])
                # pre-emit the first S^T+exp pairs of q-tile 0 so ACT starts early
                if tt >= 1:
                    for kb_pre in range((tt - 1) * 2, (tt - 1) * 2 + 2):
                        emit_s_exp(0, kb_pre)

            def emit_qt_proj(tokt):
                # compute qT for tok tile `tokt` using a shared psum slot
                pqd = psO.tile([P, 512], F32, tag="po", name="pqd")
                for cb in range(CB):
                    nc.tensor.matmul(
                        pqd[:, :], wqs[:, cb, :], xTs[:, cb, tokt * 512:(tokt + 1) * 512],
                        start=(cb == 0), stop=(cb == CB - 1))
                nc.vector.tensor_copy(qTs[:, tokt * 512:(tokt + 1) * 512], pqd[:])

            # ---- main loop: attention with deferred out-projection ----
            def emit_outproj(dep, fb, flush=False):
                # row-tiled concurrent pair: head a on array rows 0:64, head b on 64:128
                yns, q0 = dep
                if flush and fb % 2 == 1:
                    st = psS.tile([P, 1024], F32, tag="s", name="st")
                    poA, poB = st[:, 0:512], st[:, 512:1024]
                else:
                    poA = psO.tile([P, 512], F32, tag="po", name="poA")
                    poB = psO.tile([P, 512], F32, tag="po", name="poB")
                nc.tensor.matmul(poA[:, :], wos[0:64, fb * P:(fb + 1) * P],
                                 yns[0:64, :], start=True, stop=True)
                nc.tensor.matmul(poB[:, :], wos[64:128, fb * P:(fb + 1) * P],
                                 yns[64:128, :], start=True, stop=True)
                ocA = work.tile([P, 512], BF, tag="oc")
                nc.vector.tensor_copy(ocA[:], poA[:])
                nc.gpsimd.dma_start(outT0[fb * P:(fb + 1) * P, q0:q0 + 512], ocA[:])
                ocB = work.tile([P, 512], BF, tag="oc")
                nc.vector.tensor_copy(ocB[:], poB[:])
                nc.sync.dma_start(outT1[fb * P:(fb + 1) * P, q0:q0 + 512], ocB[:])

            pending = None
            for qt in range(QTS):
                q0 = qt * 512
                y0 = psY.tile([65, 512], F32, tag="y0")
                y1 = psY.tile([65, 512], F32, tag="y1")
                for kb in range(KB):
                    # emit S pairs two-at-a-time so the second pair's kT
                    # LDWEIGHTS hides under the first pair's streams
                    tgt = qt * KB + kb + LAG
                    if tgt % 2 == 0:
                        for tg in (tgt, tgt + 1):
                            if LAG <= tg < QTS * KB:
                                emit_s_exp(tg // KB, tg % KB)
                    eS = eS_store.pop((qt, kb))
                    nc.tensor.matmul(y0[:, :], Vp[:, kb, 0, 0:65], eS[:, 0:512],
                                     start=(kb == 0), stop=(kb == KB - 1))
                    nc.tensor.matmul(y1[:, :], Vp[:, kb, 1, 0:65], eS[:, 512:1024],
                                     start=(kb == 0), stop=(kb == KB - 1))
                    if pending is not None and kb % 4 == 1:
                        emit_outproj(pending, kb // 4)
                    if kb == 8 and qt + 1 < QTS:
                        emit_qt_proj(qt + 1)

                # casts release Y psum; Z rows go straight to DRAM.
                # yns rows 64:128 (head b) arrive via SBUF->SBUF DMA partition move.
                yns = work.tile([P, 512], BF, tag="yns", bufs=2)
                y1t = work.tile([64, 512], BF, tag="y1t")
                nc.vector.tensor_copy(yns[0:64, :], y0[0:64, :])
                nc.vector.tensor_copy(y1t[:], y1[0:64, :])
                nc.gpsimd.dma_start(yns[64:128, :], y1t[:])
                ztile = work.tile([65, 1024], F32, tag="zt")
                nc.vector.tensor_copy(ztile[64:65, 0:512], y0[64:65, :])
                nc.vector.tensor_copy(ztile[64:65, 512:1024], y1[64:65, :])
                nc.gpsimd.dma_start(Zt[0:1, q0:q0 + 512], ztile[64:65, 0:512])
                nc.gpsimd.dma_start(Zt[1:2, q0:q0 + 512], ztile[64:65, 512:1024])
                pending = (yns, q0)

            for fb in range(CB):
                emit_outproj(pending, fb, flush=True)

    nc.compile()
    _cached = nc
    return nc


def make_in_maps(x, w_qkv, w_out):
    """x [1,T,C] f32, w_qkv [C, 3C] f32, w_out [C, C] f32 -> per-core input dicts."""
    x = np.asarray(x, dtype=np.float32)
    w_qkv = np.asarray(w_qkv, dtype=np.float32)
    w_out = np.asarray(w_out, dtype=np.float32)
    scale = 1.0 / np.sqrt(np.float32(D))
    xT = np.ascontiguousarray(x.reshape(T, C).T).astype(BF16)  # [C, T]
    ident = np.eye(P, dtype=BF16)
    in_maps = []
    for c in range(NCORES):
        cols = slice(P * c, P * (c + 1))
        wq = np.ascontiguousarray(w_qkv[:, 0:C][:, cols] * scale).astype(BF16)
        wk = np.ascontiguousarray(w_qkv[:, C:2 * C][:, cols]).astype(BF16)
        wv = np.ascontiguousarray(w_qkv[:, 2 * C:3 * C][:, cols]).astype(BF16)
        wo = np.ascontiguousarray(w_out[P * c:P * (c + 1), :]).astype(BF16)
        in_maps.append({"xT": xT, "wq": wq, "wk": wk, "wv": wv, "wo": wo,
                        "ident": ident})
    return in_maps


def run(x, w_qkv, w_out, trace=False):
    nc = build_bass()
    in_maps = make_in_maps(x, w_qkv, w_out)
    res = run_bass_kernel_spmd(nc, in_maps, core_ids=list(range(NCORES)), trace=trace)
    acc = np.zeros((C, T), dtype=np.float32)
    for r in res.results:
        z = r["Zt"]  # [2, T]
        acc += r["outT0"].astype(np.float32) / z[0][None, :]
        acc += r["outT1"].astype(np.float32) / z[1][None, :]
    out = np.ascontiguousarray(acc.T).reshape(B, T, C)
    return out, res


def kernel(x, w_qkv, w_out):
    out, _ = run(x, w_qkv, w_out, trace=False)
    return out
